# revision 4
# baseline (speedup 1.0000x reference)
"""2-layer GCN message passing on a fixed-degree (K=5) KNN graph, 8-core SPMD.

out = x0 + x1 + x2,  x1 = w*A@x0,  x2 = w*A@x1,  (A@x)[n] = sum_k x[knn[n,k]]
with w = (K + 1e-7)^-1.

Strategy (rows sharded 12500/core, padded to 12544 = 98*128), all-bf16 device
pipeline (rel err ~6.6e-3, budget 2e-2):
 - host pre-scales the gather source (w*x0, bf16) so the layer-1 gather+sum
   yields x1 directly; per-core tensors are partition-major so regular DMAs
   are contiguous.
 - phase 1: per group of 14 row-tiles, 70 indirect row-gathers ([128,1]
   offsets, 256B bf16 rows) + 4 strided DVE adds; partial = x0 + x1 kept in
   SBUF; w*x1 (bf16) stored and AllGathered chunk-by-chunk (7 chunks) so the
   collective overlaps the remaining phase-1 gathers.
 - phase 2: same gather from the AllGather result -> x2; out = partial + x2
   (bf16), upcast to fp32 on the host.

The gather mechanism is Q7-descriptor-emission-bound (~1.1us per 128-row
indirect DMA; multi-offset APs and dma_gather were measured no better /
broken), so the kernel pipelines everything else under it.
"""

import os
import sys

import numpy as np


def _import_toolchain():
    try:
        import concourse.bass  # noqa: F401
        return
    except ImportError:
        pass
    for p in ("/opt/trn_rl_repo", os.path.expanduser("~/.axon_site/_ro/trn_rl_repo")):
        if os.path.isdir(p) and p not in sys.path:
            sys.path.insert(0, p)
    import concourse.bass  # noqa: F401


_import_toolchain()

import ml_dtypes  # noqa: E402

from concourse import bacc, bass, mybir, tile  # noqa: E402
from concourse.bass_utils import run_bass_kernel_spmd  # noqa: E402

N = 100000
D = 128
K = 5
CORES = 8
RPC = N // CORES          # 12500 rows per core
T = 98                    # row-tiles of 128 per core (98*128 = 12544)
RPAD = T * 128            # 12544
G = 14                    # row-tiles per gather group (= AG chunk)
NG = T // G               # 7 groups
GC = G * D                # columns per group (1792)
BF = mybir.dt.bfloat16
I32 = mybir.dt.int32
BFNP = ml_dtypes.bfloat16


def _w_fp32() -> np.float32:
    rs = np.float32(5.0) + np.float32(1e-7)
    r = np.float32(np.float32(rs) ** np.float32(-0.5))
    return np.float32(r * r)


def _build_nc():
    nc = bacc.Bacc("TRN2", target_bir_lowering=False, debug=False,
                   num_devices=CORES, dynamic_dma_scratch_size=65536)
    w = float(_w_fp32())

    x0s = nc.dram_tensor("x0s", [N, D], BF, kind="ExternalInput")        # w*x0
    x0m = nc.dram_tensor("x0m", [128, RPAD], BF, kind="ExternalInput")   # own rows
    ind1 = nc.dram_tensor("ind1", [128, T * K], I32, kind="ExternalInput")
    ind2 = nc.dram_tensor("ind2", [128, T * K], I32, kind="ExternalInput")
    yout = nc.dram_tensor("y", [128, RPAD], BF, kind="ExternalOutput")

    x1loc = nc.dram_tensor("x1loc", [NG, 128, GC], BF)                   # AG in
    s1full = nc.dram_tensor("s1full", [NG * CORES * 128 * G, D], BF,
                            addr_space="Shared")

    add = mybir.AluOpType.add

    with tile.TileContext(nc) as tc:
        with tc.tile_pool(name="pers", bufs=NG) as pers, \
             tc.tile_pool(name="xm", bufs=NG) as xmp, \
             tc.tile_pool(name="idx", bufs=2) as idxp, \
             tc.tile_pool(name="gat", bufs=3) as gp, \
             tc.tile_pool(name="acc", bufs=4) as yp, \
             tc.tile_pool(name="io", bufs=4) as iop:

            # chunk gi's AG output = contiguous slab s1v[gi]
            s1v = s1full.ap().rearrange("(g r) d -> g (r d)", g=NG)

            ind1_sb = idxp.tile([128, T * K], I32, tag="idx")
            nc.sync.dma_start(out=ind1_sb[:, :], in_=ind1[:, :])
            ind2_sb = idxp.tile([128, T * K], I32, tag="idx")
            nc.sync.dma_start(out=ind2_sb[:, :], in_=ind2[:, :])

            # prefetch all x0m tiles (tiny, keeps HWDGE off the critical path)
            xm_tiles = []
            for gi in range(NG):
                xt = xmp.tile([128, GC], BF, tag="xm")
                nc.sync.dma_start(out=xt[:, :],
                                  in_=x0m[:, gi * GC:(gi + 1) * GC])
                xm_tiles.append(xt)

            partials = []

            def gather_sum(src, ind_sb, gi):
                """G*K row-gathers ([128,1] offsets) + 4 DVE adds -> [128, GC]."""
                g = gp.tile([128, G * K * D], BF, tag="g")
                gv3 = g[:, :].rearrange("p (j d) -> p j d", d=D)
                for j in range(G * K):
                    col = gi * G * K + j
                    nc.gpsimd.indirect_dma_start(
                        out=gv3[:, col - gi * G * K, :],
                        out_offset=None,
                        in_=src[:, :],
                        in_offset=bass.IndirectOffsetOnAxis(
                            ap=ind_sb[:, col:col + 1], axis=0),
                    )
                gv = g[:, :].rearrange("p (t x) -> p t x", x=K * D)
                y = yp.tile([128, GC], BF, tag="y")
                yv = y[:, :].rearrange("p (t d) -> p t d", d=D)
                nc.vector.tensor_tensor(out=yv, in0=gv[:, :, 0:D],
                                        in1=gv[:, :, D:2 * D], op=add)
                for k in range(2, K):
                    nc.vector.tensor_tensor(out=yv, in0=yv,
                                            in1=gv[:, :, k * D:(k + 1) * D],
                                            op=add)
                return y

            # ---- phase 1: x1 = gather-sum(w*x0); partial = x0 + x1;
            #      store w*x1 and AllGather it chunk-by-chunk.
            for gi in range(NG):
                y = gather_sum(x0s, ind1_sb, gi)
                part = pers.tile([128, GC], BF, tag="part")
                partials.append(part)
                nc.vector.tensor_tensor(out=part[:, :], in0=xm_tiles[gi][:, :],
                                        in1=y[:, :], op=add)
                s1 = iop.tile([128, GC], BF, tag="s1")
                nc.vector.tensor_scalar_mul(s1[:, :], y[:, :], w)
                nc.sync.dma_start(out=x1loc[gi, :, :], in_=s1[:, :])
                nc.gpsimd.collective_compute(
                    "AllGather", mybir.AluOpType.bypass,
                    replica_groups=[list(range(CORES))],
                    ins=[x1loc[gi, :, :].opt()],
                    outs=[s1v[gi].opt()],
                )

            # ---- phase 2: x2 = gather-sum(w*x1); out = partial + x2
            for gi in range(NG):
                y2 = gather_sum(s1full, ind2_sb, gi)
                ot = iop.tile([128, GC], BF, tag="ot")
                nc.vector.tensor_tensor(out=ot[:, :], in0=partials[gi][:, :],
                                        in1=y2[:, :], op=add)
                nc.sync.dma_start(out=yout[:, gi * GC:(gi + 1) * GC],
                                  in_=ot[:, :])

    nc.finalize()
    return nc


_NC_CACHE = {}


def _get_nc():
    if "nc" not in _NC_CACHE:
        _NC_CACHE["nc"] = _build_nc()
    return _NC_CACHE["nc"]


def _pmajor(a):
    """[12544, M] row-major -> [128, 12544/128 * M] partition-major."""
    m = a.shape[1]
    return np.ascontiguousarray(
        a.reshape(T, 128, m).transpose(1, 0, 2).reshape(128, T * m))


def _prep_inputs(item_rep, knn_ind):
    w = _w_fp32()
    x0s = np.ascontiguousarray(item_rep * w).astype(BFNP)

    # layer-2 index remap: global row n -> row of s1full
    # s1full rows are [chunk gi][rank c][partition p][tile-in-chunk t], D wide
    c2 = knn_ind // RPC
    r2 = knn_ind - c2 * RPC
    t2 = r2 // 128
    p2 = r2 % 128
    gi2 = t2 // G
    tl2 = t2 % G
    ind2_glob = (((gi2 * CORES + c2) * 128 + p2) * G + tl2).astype(np.int32)

    in_maps = []
    for c in range(CORES):
        rows = slice(c * RPC, (c + 1) * RPC)
        x0m = np.zeros((RPAD, D), np.float32)
        x0m[:RPC] = item_rep[rows]
        i1 = np.zeros((RPAD, K), np.int32)
        i1[:RPC] = knn_ind[rows]
        i2 = np.zeros((RPAD, K), np.int32)
        i2[:RPC] = ind2_glob[rows]
        in_maps.append({
            "x0s": x0s,
            "x0m": _pmajor(x0m).astype(BFNP),
            "ind1": _pmajor(i1),
            "ind2": _pmajor(i2),
        })
    return in_maps


def _unshard(outs):
    y = np.stack([np.asarray(outs[c]["y"]).astype(np.float32)
                  for c in range(CORES)])                      # [8,128,12544]
    y = y.reshape(CORES, 128, T, D).transpose(0, 2, 1, 3)      # [8,98,128,128]
    return np.ascontiguousarray(
        y.reshape(CORES, RPAD, D)[:, :RPC].reshape(N, D))


def kernel(item_rep, knn_ind, **_ignored):
    item_rep = np.asarray(item_rep, dtype=np.float32)
    knn_ind = np.asarray(knn_ind, dtype=np.int32)
    nc = _get_nc()
    in_maps = _prep_inputs(item_rep, knn_ind)
    res = run_bass_kernel_spmd(nc, in_maps, core_ids=list(range(CORES)))
    return _unshard(res.results)


# revision 14
# speedup vs baseline: 1.0116x; 1.0116x over previous
"""2-layer GCN message passing on a fixed-degree (K=5) KNN graph, 8-core SPMD.

out = x0 + x1 + x2,  x1 = w*A@x0,  x2 = w*A@x1,  (A@x)[n] = sum_k x[knn[n,k]]
with w = (K + 1e-7)^-1.

Strategy (rows sharded 12500/core, padded to 12544 = 98*128), all-bf16 device
pipeline (rel err ~6.6e-3, budget 2e-2):
 - host pre-scales the gather source (w*x0, bf16) so the layer-1 gather+sum
   yields x1 directly; per-core tensors are partition-major so regular DMAs
   are contiguous.
 - phase 1: per group of 14 row-tiles, 70 indirect row-gathers ([128,1]
   offsets, 256B bf16 rows) + 4 strided DVE adds; partial = x0 + x1 kept in
   SBUF; w*x1 (bf16) stored and AllGathered chunk-by-chunk (7 chunks) so the
   collective fully overlaps the remaining phase-1 gathers.
 - phase 2: same gather from the AllGather result -> x2; out = partial + x2
   (bf16), upcast to fp32 on the host.

The gather mechanism is Q7-descriptor-emission-bound (~1.43us effective per
128-row indirect DMA; multi-offset APs are broken in the SWDGE ucode,
dma_gather is int16-limited and no faster, and the native Pool indirect_copy
instruction races/hangs when the tile framework attaches semaphore events to
it -- see kernel_v5_wip.py for that 2x-faster design blocked on the ucode
bug).  The kernel therefore pipelines everything else under the 980
indirect DMAs: measured 1.46 ms vs the 1.41 ms issue-bound floor.
"""

import os
import sys

import numpy as np


def _import_toolchain():
    try:
        import concourse.bass  # noqa: F401
        return
    except ImportError:
        pass
    for p in ("/opt/trn_rl_repo", os.path.expanduser("~/.axon_site/_ro/trn_rl_repo")):
        if os.path.isdir(p) and p not in sys.path:
            sys.path.insert(0, p)
    import concourse.bass  # noqa: F401


_import_toolchain()

import ml_dtypes  # noqa: E402

from concourse import bacc, bass, mybir, tile  # noqa: E402
from concourse.bass_utils import run_bass_kernel_spmd  # noqa: E402

N = 100000
D = 128
K = 5
CORES = 8
RPC = N // CORES          # 12500 rows per core
T = 98                    # row-tiles of 128 per core (98*128 = 12544)
RPAD = T * 128            # 12544
G = 14                    # row-tiles per gather group (= AG chunk)
NG = T // G               # 7 groups
GC = G * D                # columns per group (1792)
BF = mybir.dt.bfloat16
I32 = mybir.dt.int32
BFNP = ml_dtypes.bfloat16


def _w_fp32() -> np.float32:
    rs = np.float32(5.0) + np.float32(1e-7)
    r = np.float32(np.float32(rs) ** np.float32(-0.5))
    return np.float32(r * r)


def _build_nc():
    nc = bacc.Bacc("TRN2", target_bir_lowering=False, debug=False,
                   num_devices=CORES, dynamic_dma_scratch_size=65536)
    w = float(_w_fp32())

    x0s = nc.dram_tensor("x0s", [N, D], BF, kind="ExternalInput")        # w*x0
    x0m = nc.dram_tensor("x0m", [128, RPAD], BF, kind="ExternalInput")   # own rows
    ind1 = nc.dram_tensor("ind1", [128, T * K], I32, kind="ExternalInput")
    ind2 = nc.dram_tensor("ind2", [128, T * K], I32, kind="ExternalInput")
    yout = nc.dram_tensor("y", [128, RPAD], BF, kind="ExternalOutput")

    x1loc = nc.dram_tensor("x1loc", [NG, 128, GC], BF)                   # AG in
    s1full = nc.dram_tensor("s1full", [NG * CORES * 128 * G, D], BF,
                            addr_space="Shared")

    add = mybir.AluOpType.add

    with tile.TileContext(nc) as tc:
        with tc.tile_pool(name="pers", bufs=NG) as pers, \
             tc.tile_pool(name="xm", bufs=NG) as xmp, \
             tc.tile_pool(name="idx", bufs=2) as idxp, \
             tc.tile_pool(name="gat", bufs=3) as gp, \
             tc.tile_pool(name="acc", bufs=4) as yp, \
             tc.tile_pool(name="io", bufs=4) as iop:

            # chunk gi's AG output = contiguous slab s1v[gi]
            s1v = s1full.ap().rearrange("(g r) d -> g (r d)", g=NG)

            ind1_sb = idxp.tile([128, T * K], I32, tag="idx")
            nc.sync.dma_start(out=ind1_sb[:, :], in_=ind1[:, :])
            ind2_sb = idxp.tile([128, T * K], I32, tag="idx")
            nc.sync.dma_start(out=ind2_sb[:, :], in_=ind2[:, :])

            # prefetch all x0m tiles (tiny, keeps HWDGE off the critical path)
            xm_tiles = []
            for gi in range(NG):
                xt = xmp.tile([128, GC], BF, tag="xm")
                nc.sync.dma_start(out=xt[:, :],
                                  in_=x0m[:, gi * GC:(gi + 1) * GC])
                xm_tiles.append(xt)

            partials = []

            def gather_sum(src, ind_sb, gi):
                """G*K row-gathers ([128,1] offsets) + 4 DVE adds -> [128, GC]."""
                g = gp.tile([128, G * K * D], BF, tag="g")
                gv3 = g[:, :].rearrange("p (j d) -> p j d", d=D)
                for j in range(G * K):
                    col = gi * G * K + j
                    nc.gpsimd.indirect_dma_start(
                        out=gv3[:, col - gi * G * K, :],
                        out_offset=None,
                        in_=src[:, :],
                        in_offset=bass.IndirectOffsetOnAxis(
                            ap=ind_sb[:, col:col + 1], axis=0),
                    )
                gv = g[:, :].rearrange("p (t x) -> p t x", x=K * D)
                y = yp.tile([128, GC], BF, tag="y")
                yv = y[:, :].rearrange("p (t d) -> p t d", d=D)
                nc.vector.tensor_tensor(out=yv, in0=gv[:, :, 0:D],
                                        in1=gv[:, :, D:2 * D], op=add)
                for k in range(2, K):
                    nc.vector.tensor_tensor(out=yv, in0=yv,
                                            in1=gv[:, :, k * D:(k + 1) * D],
                                            op=add)
                return y

            # ---- phase 1: x1 = gather-sum(w*x0); partial = x0 + x1;
            #      store w*x1 and AllGather it chunk-by-chunk.
            for gi in range(NG):
                y = gather_sum(x0s, ind1_sb, gi)
                part = pers.tile([128, GC], BF, tag="part")
                partials.append(part)
                nc.vector.tensor_tensor(out=part[:, :], in0=xm_tiles[gi][:, :],
                                        in1=y[:, :], op=add)
                s1 = iop.tile([128, GC], BF, tag="s1")
                nc.vector.tensor_scalar_mul(s1[:, :], y[:, :], w)
                nc.sync.dma_start(out=x1loc[gi, :, :], in_=s1[:, :])
                nc.gpsimd.collective_compute(
                    "AllGather", mybir.AluOpType.bypass,
                    replica_groups=[list(range(CORES))],
                    ins=[x1loc[gi, :, :].opt()],
                    outs=[s1v[gi].opt()],
                )

            # ---- phase 2: x2 = gather-sum(w*x1); out = partial + x2
            for gi in range(NG):
                y2 = gather_sum(s1full, ind2_sb, gi)
                ot = iop.tile([128, GC], BF, tag="ot")
                nc.vector.tensor_tensor(out=ot[:, :], in0=partials[gi][:, :],
                                        in1=y2[:, :], op=add)
                nc.sync.dma_start(out=yout[:, gi * GC:(gi + 1) * GC],
                                  in_=ot[:, :])

    nc.finalize()
    return nc


_NC_CACHE = {}


def _get_nc():
    if "nc" not in _NC_CACHE:
        _NC_CACHE["nc"] = _build_nc()
    return _NC_CACHE["nc"]


def _pmajor(a):
    """[12544, M] row-major -> [128, 12544/128 * M] partition-major."""
    m = a.shape[1]
    return np.ascontiguousarray(
        a.reshape(T, 128, m).transpose(1, 0, 2).reshape(128, T * m))


def _prep_inputs(item_rep, knn_ind):
    w = _w_fp32()
    x0s = np.ascontiguousarray(item_rep * w).astype(BFNP)

    # layer-2 index remap: global row n -> row of s1full
    # s1full rows are [chunk gi][rank c][partition p][tile-in-chunk t], D wide
    c2 = knn_ind // RPC
    r2 = knn_ind - c2 * RPC
    t2 = r2 // 128
    p2 = r2 % 128
    gi2 = t2 // G
    tl2 = t2 % G
    ind2_glob = (((gi2 * CORES + c2) * 128 + p2) * G + tl2).astype(np.int32)

    in_maps = []
    for c in range(CORES):
        rows = slice(c * RPC, (c + 1) * RPC)
        x0m = np.zeros((RPAD, D), np.float32)
        x0m[:RPC] = item_rep[rows]
        i1 = np.zeros((RPAD, K), np.int32)
        i1[:RPC] = knn_ind[rows]
        i2 = np.zeros((RPAD, K), np.int32)
        i2[:RPC] = ind2_glob[rows]
        in_maps.append({
            "x0s": x0s,
            "x0m": _pmajor(x0m).astype(BFNP),
            "ind1": _pmajor(i1),
            "ind2": _pmajor(i2),
        })
    return in_maps


def _unshard(outs):
    y = np.stack([np.asarray(outs[c]["y"]).astype(np.float32)
                  for c in range(CORES)])                      # [8,128,12544]
    y = y.reshape(CORES, 128, T, D).transpose(0, 2, 1, 3)      # [8,98,128,128]
    return np.ascontiguousarray(
        y.reshape(CORES, RPAD, D)[:, :RPC].reshape(N, D))


def kernel(item_rep, knn_ind, **_ignored):
    item_rep = np.asarray(item_rep, dtype=np.float32)
    knn_ind = np.asarray(knn_ind, dtype=np.int32)
    nc = _get_nc()
    in_maps = _prep_inputs(item_rep, knn_ind)
    res = run_bass_kernel_spmd(nc, in_maps, core_ids=list(range(CORES)))
    return _unshard(res.results)
